# revision 2
# baseline (speedup 1.0000x reference)
"""Trainium2 Bass kernel: 3x3 SAME conv (stride 1), NCHW fp32.

Problem: image [32, 64, 112, 112] * weight [64, 64, 3, 3] + bias [64]
Sharding: data-parallel over batch across 8 NeuronCores (4 images each).

Per-core strategy (pixel-major matmuls):
  - The padded image (114x114 per channel, bf16) lives flat in SBUF as
    [128, L]: partition 64*s + cin holds channel cin's flat padded pixels
    shifted by s*114 (s=1 = one padded row down). The two shifted copies
    make the matmul contraction dim K = 128 = (cin, row-shift).
  - GEMM orientation: lhsT (stationary) = image patch [K=128, M=128
    output pixels], rhs (moving) = weights [K=128, F=64 couts], out =
    PSUM [128 pixels, 64 couts]. Output pixels are indexed o = h*114 + w
    over the padded width (w=112,113 are garbage, dropped on host).
  - 6 accumulating matmuls per 128-pixel block cover the 9 taps:
    j=0..2: offset o+j      -> taps (kh=0,kw=j) on s=0 rows and
                               (kh=1,kw=j) on s=1 rows (K=128 pair);
    j=3..5: offset o+228+kw -> tap (kh=2,kw) on s=0 rows; the s=1 rows
                               see garbage data x zero weight rows.
  - 8 blocks accumulate into one 2KB PSUM bank ([128, 512] f32); DVE
    drains each bank with a cast to bf16, and one DMA per bank writes
    HBM in [n, pixel%128, block, cout] order (1KB contiguous runs per
    partition). Host reassembles NCHW, drops the garbage columns, adds
    bias, and converts to f32.
  - Host-side work (pad/shift-copy prep, transpose, bias) keeps the
    device graph to: input DMA 13.4MB, 2400 matmuls of 64 output rows,
    52 DVE drains, 6.7MB output DMA per core.
"""

import numpy as np

import concourse.bass as bass
import concourse.mybir as mybir
import concourse.tile as tile
from concourse import bacc, bass_utils

N_CORES = 8
IMGS = 4  # images per core
CIN = 64
COUT = 64
H = 112
W = 112
HP = H + 2  # 114
WP = W + 2  # 114
CH_LEN = HP * WP  # 12996 padded flat length per channel
NPIX = H * WP  # 12768 output pixel slots (incl. 2 garbage cols per row)
MBLK = 128  # output pixels per block (PSUM partitions)
NBLK = 100  # blocks per image (100*128 = 12800 >= 12768)
GRP = 8  # blocks per PSUM bank (8*64 f32 = 2KB)
NGRP = 13  # 12 full groups + 1 tail group of 4
TAIL_G = NBLK - GRP * (NGRP - 1)  # 4
N_CHUNK = 4  # input DMA chunks per image
L_BUF = 13056  # SBUF cols per partition (>= 99*128+230+127+1, 4*3264)
L_CHUNK = L_BUF // N_CHUNK  # 3264
TOTAL_IN = 832000  # 64*12996 + 256 zero tail (covers shifted-copy reads)
OUT_ROW = NBLK * COUT  # 6400 out elems per partition-row per image
OUT_LEN = 128 * OUT_ROW  # 819200 per image
# matmul lhsT column offsets per block base: pairs (kh=0/1, kw) then
# singles (kh=2, kw) read via the s=0 rows at +2 padded rows.
TAP_OFFS = (0, 1, 2, 2 * WP, 2 * WP + 1, 2 * WP + 2)

F32 = mybir.dt.float32
BF16 = mybir.dt.bfloat16


def _ap(ap_obj, offset, dims):
    """Manual AP on the same tensor handle; dims = [[step, count], ...]."""
    return bass.AP(tensor=ap_obj.tensor, offset=offset, ap=dims)


def build_nc(n_imgs=IMGS):
    nc = bacc.Bacc(
        "TRN2",
        target_bir_lowering=False,
        debug=False,
        num_devices=N_CORES,
    )
    img_d = nc.dram_tensor("image_flat", [n_imgs, TOTAL_IN], BF16, kind="ExternalInput")
    wt_d = nc.dram_tensor("weight6", [128, 6 * COUT], BF16, kind="ExternalInput")
    out_d = nc.dram_tensor("out", [n_imgs, OUT_LEN], BF16, kind="ExternalOutput")

    img_ap = img_d.ap()
    out_ap = out_d.ap()

    with tile.TileContext(nc) as tc:
        with (
            tc.tile_pool(name="img", bufs=2) as img_pool,
            tc.tile_pool(name="wt", bufs=1) as wt_pool,
            tc.tile_pool(name="stage", bufs=4) as stage_pool,
            tc.tile_pool(name="psum", bufs=4, space="PSUM") as psum_pool,
        ):
            wt_t = wt_pool.tile([128, 6 * COUT], BF16)
            nc.sync.dma_start(wt_t[:], wt_d.ap()[:])

            for n in range(n_imgs):
                img_t = img_pool.tile([128, L_BUF], BF16)
                # partition 64*s + cin <- flat padded channel cin shifted by
                # s*114; chunked so drain DMAs can interleave.
                for c in range(N_CHUNK):
                    src = _ap(
                        img_ap,
                        n * TOTAL_IN + c * L_CHUNK,
                        [[WP, 2], [CH_LEN, CIN], [1, L_CHUNK]],
                    )
                    nc.sync.dma_start(
                        img_t[:, c * L_CHUNK : (c + 1) * L_CHUNK], src
                    )

                for g in range(NGRP):
                    gsz = GRP if g < NGRP - 1 else TAIL_G
                    ps = psum_pool.tile([128, GRP * COUT], F32)
                    for i in range(gsz):
                        base = (g * GRP + i) * MBLK
                        dst = ps[:, i * COUT : (i + 1) * COUT]
                        for j, off in enumerate(TAP_OFFS):
                            nc.tensor.matmul(
                                dst,
                                img_t[:, base + off : base + off + MBLK],
                                wt_t[:, j * COUT : (j + 1) * COUT],
                                start=(j == 0),
                                stop=(j == 5),
                                skip_group_check=True,
                            )
                    stg = stage_pool.tile([128, GRP * COUT], BF16)
                    nc.vector.tensor_scalar_add(
                        stg[:, : gsz * COUT], ps[:, : gsz * COUT], 0.0
                    )
                    dst = _ap(
                        out_ap,
                        n * OUT_LEN + g * GRP * COUT,
                        [[OUT_ROW, 128], [1, gsz * COUT]],
                    )
                    nc.sync.dma_start(dst, stg[:, : gsz * COUT])

    nc.compile()
    return nc


_NC_CACHE = {}


def _get_nc(n_imgs=IMGS):
    if n_imgs not in _NC_CACHE:
        _NC_CACHE[n_imgs] = build_nc(n_imgs)
    return _NC_CACHE[n_imgs]


def _prep_inputs(image, weight):
    import ml_dtypes

    bf16 = ml_dtypes.bfloat16
    image = np.asarray(image, dtype=np.float32)
    weight = np.asarray(weight, dtype=np.float32)
    n = image.shape[0]
    pad = np.zeros((n, CIN, HP, WP), np.float32)
    pad[:, :, 1 : 1 + H, 1 : 1 + W] = image
    img_flat = np.zeros((n, TOTAL_IN), bf16)
    img_flat[:, : CIN * CH_LEN] = pad.reshape(n, CIN * CH_LEN).astype(bf16)
    # weight blocks [128, 6*COUT]: j=0..2 pairs (kh=0 lower / kh=1 upper
    # rows, kw=j); j=3..5 singles (kh=2, kw=j-3) on lower rows only.
    wt6 = np.zeros((128, 6, COUT), np.float32)
    for j in range(3):
        wt6[:CIN, j] = weight[:, :, 0, j].T
        wt6[CIN:, j] = weight[:, :, 1, j].T
        wt6[:CIN, 3 + j] = weight[:, :, 2, j].T
    return img_flat, np.ascontiguousarray(wt6.reshape(128, 6 * COUT)).astype(bf16)


def run_cores(image, weight, bias, trace=False, **kw):
    """Shard over 8 cores, run, return (full_output, BassKernelResults)."""
    img_flat, wt6 = _prep_inputs(image, weight)
    n = img_flat.shape[0]
    per = n // N_CORES
    assert per * N_CORES == n
    nc = _get_nc(per)
    in_maps = [
        {
            "image_flat": np.ascontiguousarray(img_flat[i * per : (i + 1) * per]),
            "weight6": wt6,
        }
        for i in range(N_CORES)
    ]
    res = bass_utils.run_bass_kernel_spmd(
        nc, in_maps, core_ids=list(range(N_CORES)), trace=trace, **kw
    )
    outs = []
    bias32 = np.asarray(bias, dtype=np.float32)
    for i in range(N_CORES):
        arr = np.asarray(res.results[i]["out"]).reshape(per, 128, NBLK, COUT)
        # pixel o = block*128 + p  ->  [n, cout, o]
        pc = arr.transpose(0, 3, 2, 1).reshape(per, COUT, NBLK * 128)
        img = pc[:, :, :NPIX].reshape(per, COUT, H, WP)[:, :, :, :W]
        outs.append(img.astype(np.float32) + bias32[None, :, None, None])
    return np.concatenate(outs, axis=0), res


def kernel(image, weight, bias):
    out, _ = run_cores(image, weight, bias, trace=False)
    return out
